# revision 2
# baseline (speedup 1.0000x reference)
"""Trainium2 Bass kernel: adaptive-input softmax ('softmax' mode), 8 NeuronCores.

Strategy: vocab tensor-parallel. Each core owns a 1/8 slice of the head token
columns (2500 of 20000), tail0 columns (2500 of 20000) and tail1 columns
(1283 of ceil(10257/8)*8, zero-padded), computes partition-local logits ->
exp, and the per-row softmax denominators are completed with a small
cross-core AllReduce of per-row exp-sums (overlapped with compute).
The 2 cluster logits are computed replicated on every core.

All matmuls run in bf16 (inputs pre-cast on host); exp/normalization in f32.
"""
import numpy as np
import ml_dtypes
from contextlib import ExitStack

import concourse.bass as bass
import concourse.tile as tile
from concourse import bacc, mybir
from concourse.bass_utils import run_bass_kernel_spmd

N_CORES = 8
D = 1024
KT = D // 128                      # contraction k-tiles over D
B0 = 20000                         # head token columns
HEAD_SLICE = B0 // N_CORES         # 2500 per core
T0_ALL = 20000
T0_SLICE = T0_ALL // N_CORES       # 2500 per core
T1_ALL = 10257
T1_SLICE = -(-T1_ALL // N_CORES)   # 1283 per core (global pad to 10264)
T1_PADDED = T1_SLICE * N_CORES
HEAD_COLS = HEAD_SLICE + 2         # + 2 replicated cluster columns
OUT_COLS = HEAD_SLICE + T0_SLICE + T1_SLICE   # 6283 per-core output columns
P0 = 256                           # tail0 projection dim
P1 = 64                            # tail1 projection dim
V = B0 + T0_ALL + T1_ALL           # 50257

BF16 = mybir.dt.bfloat16
F32 = mybir.dt.float32
EXP = mybir.ActivationFunctionType.Exp
ADD = mybir.AluOpType.add
AX = mybir.AxisListType.X


def _tiles(total, step=512):
    out, off = [], 0
    while off < total:
        w = min(step, total - off)
        out.append((off, w))
        off += w
    return out


def build(rows):
    assert rows % 256 == 0
    m_tiles = rows // 128

    nc = bacc.Bacc("TRN2", target_bir_lowering=False, debug=False,
                   num_devices=N_CORES)
    xT_ext = nc.declare_dram_parameter("xT", [D, rows], BF16, isOutput=False)
    wh_ext = nc.declare_dram_parameter("wh", [D, HEAD_COLS], BF16, isOutput=False)
    p0_ext = nc.declare_dram_parameter("p0", [D, P0], BF16, isOutput=False)
    w0_ext = nc.declare_dram_parameter("w0", [P0, T0_SLICE], BF16, isOutput=False)
    p1_ext = nc.declare_dram_parameter("p1", [D, P1], BF16, isOutput=False)
    w1_ext = nc.declare_dram_parameter("w1", [P1, T1_SLICE], BF16, isOutput=False)
    npad_ext = nc.declare_dram_parameter("negpad", [128, 1], F32, isOutput=False)
    out_ext = nc.declare_dram_parameter("out", [rows, OUT_COLS], F32, isOutput=True)

    with ExitStack() as ctx:
        tc = ctx.enter_context(tile.TileContext(nc))
        const = ctx.enter_context(tc.tile_pool(name="const", bufs=1))
        psum_pool = ctx.enter_context(tc.tile_pool(name="psum", bufs=6, space="PSUM"))
        exp_pool = ctx.enter_context(tc.tile_pool(name="exppool", bufs=4))
        outp = ctx.enter_context(tc.tile_pool(name="outp", bufs=8))
        small = ctx.enter_context(tc.tile_pool(name="small", bufs=6))
        dram = ctx.enter_context(tc.tile_pool(name="dram", bufs=2, space="DRAM"))

        # ---------- resident inputs ----------
        xT_sb = const.tile([128, KT, rows], BF16, name="xT_sb")
        wh_sb = const.tile([128, KT, HEAD_COLS], BF16, name="wh_sb")
        p0_sb = const.tile([128, KT, P0], BF16, name="p0_sb")
        p1_sb = const.tile([128, KT, P1], BF16, name="p1_sb")
        w0_sb = const.tile([128, P0 // 128, T0_SLICE], BF16, name="w0_sb")
        w1_sb = const.tile([P1, T1_SLICE], BF16, name="w1_sb")
        npad_sb = const.tile([128, 1], F32, name="npad_sb")
        for k in range(KT):
            nc.sync.dma_start(out=xT_sb[:, k, :], in_=xT_ext[k * 128:(k + 1) * 128, :])
            nc.sync.dma_start(out=wh_sb[:, k, :], in_=wh_ext[k * 128:(k + 1) * 128, :])
            nc.sync.dma_start(out=p0_sb[:, k, :], in_=p0_ext[k * 128:(k + 1) * 128, :])
            nc.sync.dma_start(out=p1_sb[:, k, :], in_=p1_ext[k * 128:(k + 1) * 128, :])
        for k in range(P0 // 128):
            nc.sync.dma_start(out=w0_sb[:, k, :], in_=w0_ext[k * 128:(k + 1) * 128, :])
        nc.sync.dma_start(out=w1_sb[:, :], in_=w1_ext[:, :])
        nc.sync.dma_start(out=npad_sb[:], in_=npad_ext[:])

        # ---------- tail hidden projections: h0T = p0^T x^T, h1T = p1^T x^T ----
        h0T_sb = const.tile([128, P0 // 128, rows], BF16, name="h0T_sb")
        h1T_sb = const.tile([P1, rows], BF16, name="h1T_sb")
        for (roff, rw) in _tiles(rows, 512):
            for mp in range(P0 // 128):
                ps = psum_pool.tile([128, 512], F32, name="ps")
                for k in range(KT):
                    nc.tensor.matmul(ps[:, :rw],
                                     lhsT=p0_sb[:, k, mp * 128:(mp + 1) * 128],
                                     rhs=xT_sb[:, k, roff:roff + rw],
                                     start=(k == 0), stop=(k == KT - 1))
                nc.vector.tensor_copy(h0T_sb[:, mp, roff:roff + rw], ps[:, :rw])
            ps = psum_pool.tile([128, 512], F32, name="ps")
            for k in range(KT):
                nc.tensor.matmul(ps[:P1, :rw], lhsT=p1_sb[:, k, :],
                                 rhs=xT_sb[:, k, roff:roff + rw],
                                 start=(k == 0), stop=(k == KT - 1))
            nc.vector.tensor_copy(h1T_sb[:, roff:roff + rw], ps[:P1, :rw])

        # ---------- main loop over row tiles (128 rows each) ----------
        head_tiles = _tiles(HEAD_COLS)        # last tile contains 2 cluster cols
        t0_tiles = _tiles(T0_SLICE)
        t1_tiles = _tiles(T1_SLICE)
        nh, n0, n1 = len(head_tiles), len(t0_tiles), len(t1_tiles)

        blocks = [list(range(b, b + 2)) for b in range(0, m_tiles, 2)]
        for blk in blocks:
            per_m = {}
            for m in blk:
                r0 = m * 128
                exph = exp_pool.tile([128, HEAD_SLICE], BF16, name="exph")
                expt0 = exp_pool.tile([128, T0_SLICE], BF16, name="expt0")
                expt1 = exp_pool.tile([128, T1_SLICE], BF16, name="expt1")
                partials = small.tile([128, nh + n0 + n1], F32, name="partials")
                cexp = small.tile([128, 2], F32, name="cexp")
                sums = small.tile([128, 3], F32, name="sums")

                pcol = 0
                for ti, (off, w) in enumerate(head_tiles):
                    ps = psum_pool.tile([128, 512], F32, name="ps")
                    for k in range(KT):
                        nc.tensor.matmul(ps[:, :w], lhsT=xT_sb[:, k, r0:r0 + 128],
                                         rhs=wh_sb[:, k, off:off + w],
                                         start=(k == 0), stop=(k == KT - 1))
                    if ti == nh - 1:
                        wt = w - 2   # exclude the 2 cluster cols from sum/output
                        nc.scalar.activation(exph[:, off:off + wt], ps[:, :wt], EXP,
                                             accum_out=partials[:, pcol:pcol + 1])
                        nc.scalar.activation(cexp[:, :], ps[:, wt:w], EXP)
                    else:
                        nc.scalar.activation(exph[:, off:off + w], ps[:, :w], EXP,
                                             accum_out=partials[:, pcol:pcol + 1])
                    pcol += 1
                for (off, w) in t0_tiles:
                    ps = psum_pool.tile([128, 512], F32, name="ps")
                    for k in range(P0 // 128):
                        nc.tensor.matmul(ps[:, :w], lhsT=h0T_sb[:, k, r0:r0 + 128],
                                         rhs=w0_sb[:, k, off:off + w],
                                         start=(k == 0), stop=(k == P0 // 128 - 1))
                    nc.scalar.activation(expt0[:, off:off + w], ps[:, :w], EXP,
                                         accum_out=partials[:, pcol:pcol + 1])
                    pcol += 1
                for (off, w) in t1_tiles:
                    ps = psum_pool.tile([128, 512], F32, name="ps")
                    nc.tensor.matmul(ps[:, :w], lhsT=h1T_sb[:, r0:r0 + 128],
                                     rhs=w1_sb[:, off:off + w],
                                     start=True, stop=True)
                    nc.scalar.activation(expt1[:, off:off + w], ps[:, :w], EXP,
                                         accum_out=partials[:, pcol:pcol + 1])
                    pcol += 1

                t1raw = small.tile([128, 1], F32, name="t1raw")
                nc.vector.tensor_reduce(out=sums[:, 0:1], in_=partials[:, 0:nh],
                                        axis=AX, op=ADD)
                nc.vector.tensor_reduce(out=sums[:, 1:2], in_=partials[:, nh:nh + n0],
                                        axis=AX, op=ADD)
                nc.vector.tensor_reduce(out=t1raw[:], in_=partials[:, nh + n0:nh + n0 + n1],
                                        axis=AX, op=ADD)
                nc.vector.tensor_add(sums[:, 2:3], t1raw[:], npad_sb[:])
                per_m[m] = (exph, expt0, expt1, cexp, sums)

            # one AllReduce per block: per-row [head_sum, t0_sum, t1_sum]
            bs = len(blk)
            cc_in = dram.tile([bs * 128, 3], F32, name="cc_in")
            cc_out = dram.tile([bs * 128, 3], F32, name="cc_out", addr_space="Shared")
            for i, m in enumerate(blk):
                nc.sync.dma_start(out=cc_in[i * 128:(i + 1) * 128, :],
                                  in_=per_m[m][4][:, :])
            nc.gpsimd.collective_compute(
                "AllReduce", ADD,
                replica_groups=[list(range(N_CORES))],
                ins=[cc_in.opt()], outs=[cc_out.opt()],
            )
            for i, m in enumerate(blk):
                exph, expt0, expt1, cexp, sums = per_m[m]
                r0 = m * 128
                gs = small.tile([128, 3], F32, name="gs")
                nc.sync.dma_start(out=gs[:], in_=cc_out[i * 128:(i + 1) * 128, :])
                scl = small.tile([128, 10], F32, name="scl")
                # 0: cexp0+cexp1          1: head_den    2: 1/head_den
                # 3: 1/t0_den             4: 1/t1_den
                # 5: cexp0/head_den       6: tail0 scale
                # 7: cexp1/head_den       8: tail1 scale
                nc.vector.tensor_reduce(out=scl[:, 0:1], in_=cexp[:, 0:2], axis=AX, op=ADD)
                nc.vector.tensor_add(scl[:, 1:2], gs[:, 0:1], scl[:, 0:1])
                nc.vector.reciprocal(scl[:, 2:3], scl[:, 1:2])
                nc.vector.reciprocal(scl[:, 3:4], gs[:, 1:2])
                nc.vector.reciprocal(scl[:, 4:5], gs[:, 2:3])
                nc.vector.tensor_mul(scl[:, 5:6], cexp[:, 0:1], scl[:, 2:3])
                nc.vector.tensor_mul(scl[:, 6:7], scl[:, 5:6], scl[:, 3:4])
                nc.vector.tensor_mul(scl[:, 7:8], cexp[:, 1:2], scl[:, 2:3])
                nc.vector.tensor_mul(scl[:, 8:9], scl[:, 7:8], scl[:, 4:5])

                for (off, w) in _tiles(HEAD_SLICE):
                    ot = outp.tile([128, 512], F32, name="ot")
                    nc.vector.tensor_scalar_mul(ot[:, :w], exph[:, off:off + w],
                                                scl[:, 2:3])
                    nc.sync.dma_start(out=out_ext[r0:r0 + 128, off:off + w],
                                      in_=ot[:, :w])
                for (off, w) in t0_tiles:
                    ot = outp.tile([128, 512], F32, name="ot")
                    nc.vector.tensor_scalar_mul(ot[:, :w], expt0[:, off:off + w],
                                                scl[:, 6:7])
                    nc.sync.dma_start(
                        out=out_ext[r0:r0 + 128, HEAD_SLICE + off:HEAD_SLICE + off + w],
                        in_=ot[:, :w])
                base = HEAD_SLICE + T0_SLICE
                for (off, w) in t1_tiles:
                    ot = outp.tile([128, 512], F32, name="ot")
                    nc.vector.tensor_scalar_mul(ot[:, :w], expt1[:, off:off + w],
                                                scl[:, 8:9])
                    nc.sync.dma_start(
                        out=out_ext[r0:r0 + 128, base + off:base + off + w],
                        in_=ot[:, :w])

    nc.compile()
    return nc


def _shard_inputs(x2d, head_weight, tail_proj_0, tail_w_0, tail_proj_1, tail_w_1):
    bf = ml_dtypes.bfloat16
    xT = np.ascontiguousarray(x2d.T).astype(bf)
    cluster = head_weight[:, B0:B0 + 2]
    w1p = np.zeros((P1, T1_PADDED), np.float32)
    w1p[:, :T1_ALL] = tail_w_1
    p0b = np.ascontiguousarray(tail_proj_0).astype(bf)
    p1b = np.ascontiguousarray(tail_proj_1).astype(bf)
    in_maps = []
    for c in range(N_CORES):
        wh = np.concatenate(
            [head_weight[:, c * HEAD_SLICE:(c + 1) * HEAD_SLICE], cluster], axis=1)
        npad = -float(T1_PADDED - T1_ALL) if c == N_CORES - 1 else 0.0
        in_maps.append({
            "xT": xT,
            "wh": np.ascontiguousarray(wh).astype(bf),
            "p0": p0b,
            "w0": np.ascontiguousarray(
                tail_w_0[:, c * T0_SLICE:(c + 1) * T0_SLICE]).astype(bf),
            "p1": p1b,
            "w1": np.ascontiguousarray(
                w1p[:, c * T1_SLICE:(c + 1) * T1_SLICE]).astype(bf),
            "negpad": np.full((128, 1), npad, np.float32),
        })
    return in_maps


def _assemble(outs, rows):
    full = np.empty((rows, V), np.float32)
    for c in range(N_CORES):
        o = outs[c]
        full[:, c * HEAD_SLICE:(c + 1) * HEAD_SLICE] = o[:, :HEAD_SLICE]
        full[:, B0 + c * T0_SLICE:B0 + (c + 1) * T0_SLICE] = \
            o[:, HEAD_SLICE:HEAD_SLICE + T0_SLICE]
        lo = c * T1_SLICE
        hi = min((c + 1) * T1_SLICE, T1_ALL)
        full[:, B0 + T0_ALL + lo:B0 + T0_ALL + hi] = \
            o[:, HEAD_SLICE + T0_SLICE:HEAD_SLICE + T0_SLICE + (hi - lo)]
    return full


RUN_KWARGS = {}      # test harness may set e.g. {"trace": True}
LAST_RESULT = None   # test harness reads exec_time_ns / profile from here


def kernel(x, head_weight, tail_proj_0, tail_w_0, tail_proj_1, tail_w_1):
    global LAST_RESULT
    x = np.asarray(x, dtype=np.float32)
    n, t, d = x.shape
    rows = n * t
    nc = build(rows)
    in_maps = _shard_inputs(
        x.reshape(rows, d),
        np.asarray(head_weight, dtype=np.float32),
        np.asarray(tail_proj_0, dtype=np.float32),
        np.asarray(tail_w_0, dtype=np.float32),
        np.asarray(tail_proj_1, dtype=np.float32),
        np.asarray(tail_w_1, dtype=np.float32),
    )
    res = run_bass_kernel_spmd(nc, in_maps, core_ids=list(range(N_CORES)),
                               **RUN_KWARGS)
    LAST_RESULT = res
    full = _assemble([r["out"] for r in res.results], rows)
    return full.reshape(n, t, V)
